# revision 3
# baseline (speedup 1.0000x reference)
"""Trainium2 Bass kernel for nn_MMHA_78039555768536.

Gated mix of per-segment causal softmax attention and a linear-attention
memory (delta rule, memory summed over batch per segment).

Strategy (8 cores): reformulate the memory recurrence as a linear matrix
recurrence  M_{t+1} = A_t M_t + B_t  with
    A_t = I - sum_b sk_b^T diag(1/d_b) sk_b   (symmetric A-part)
    B_t = sum_b sk_b^T v_b
    d_b = sk_b @ z_{b,t};  z is a prefix of column-sums of sk (M-independent)
Core c owns segments {2c, 2c+1} for all batches.  Two all-gathers:
 AG1: per-segment colsums of sk (for the z prefix)  [tiny]
 AG2: per-core pair composition (Abar^T, Bbar)      [1 MB bf16 per rank]
Then every core redundantly runs the 7-step pair chain and selects its own
prefix M via a per-core one-hot input (SPMD, no branches).

All matmul operands are bf16 (fp32 PSUM accumulation); validated vs the
fp32 reference at ~3e-3 relative-to-absmax error in a numpy prototype.
"""

import os
import sys

sys.path.insert(0, "/opt/trn_rl_repo")

STAGE = int(os.environ.get("KSTAGE", "9"))
SUB = int(os.environ.get("KSUB", "9"))
SIMSAFE = int(os.environ.get("KSIMSAFE", "0"))  # CoreSim rejects PSUM re-accumulate

from contextlib import ExitStack

import numpy as np
import ml_dtypes

import concourse.bass as bass
import concourse.bacc as bacc
import concourse.tile as tile
from concourse import mybir
from concourse import bass_utils

B, L, DIN = 4, 8192, 512
H, D, SEG = 8, 64, 512
HD = H * D
NSEG = L // SEG          # 16
NC = 8                   # cores
SPC = NSEG // NC         # segments per core = 2
P = 128
NB = HD // P             # 4 blocks of 128
BS = B * SPC             # batch-segment units per core = 8

bf = mybir.dt.bfloat16
f32 = mybir.dt.float32
AF = mybir.ActivationFunctionType
OP = mybir.AluOpType
bf_np = ml_dtypes.bfloat16

_CACHE = {}


def _build():
    nc = bacc.Bacc(
        "TRN2",
        target_bir_lowering=False,
        debug=False,
        enable_asserts=False,
        num_devices=NC,
    )

    # ---------------- DRAM I/O ----------------
    xt_d = nc.dram_tensor("xt", [B, SPC, NB, P, SEG], bf, kind="ExternalInput").ap()
    wq_d = nc.dram_tensor("wq", [NB, P, HD], bf, kind="ExternalInput").ap()
    wk_d = nc.dram_tensor("wk", [NB, P, HD], bf, kind="ExternalInput").ap()
    wv_d = nc.dram_tensor("wv", [NB, P, HD], bf, kind="ExternalInput").ap()
    wd_d = nc.dram_tensor("wd", [NB, P, D], bf, kind="ExternalInput").ap()
    gcol_d = nc.dram_tensor("gcol", [P, NB], f32, kind="ExternalInput").ap()
    omg_d = nc.dram_tensor("omg", [P, NB], f32, kind="ExternalInput").ap()
    zmask_d = nc.dram_tensor("zmask", [64, NC], f32, kind="ExternalInput").ap()
    oh_d = nc.dram_tensor("oh", [P, NC], f32, kind="ExternalInput").ap()
    mask_d = nc.dram_tensor("cmask", [P, P], bf, kind="ExternalInput").ap()
    ident_d = nc.dram_tensor("ident", [P, P], bf, kind="ExternalInput").ap()
    out_d = nc.dram_tensor("out", [B, SPC, SEG, D], f32, kind="ExternalOutput").ap()

    with tile.TileContext(nc) as tc, ExitStack() as ctx:
        # ---------------- constant / DRAM pools ----------------
        const = ctx.enter_context(tc.tile_pool(name="const", bufs=1))
        dram = ctx.enter_context(tc.tile_pool(name="dram", bufs=1, space="DRAM"))
        keep = ctx.enter_context(tc.tile_pool(name="keep", bufs=BS))
        phb = ctx.enter_context(tc.tile_pool(name="phb", bufs=1))  # phase-B singles

        WQ = const.tile([P, NB, HD], bf)
        WK = const.tile([P, NB, HD], bf)
        WV = const.tile([P, NB, HD], bf)
        WD = const.tile([P, NB, D], bf)
        GC = const.tile([P, NB], f32)
        OMG = const.tile([P, NB], f32)
        ZM = const.tile([64, NC], f32)
        OH = const.tile([P, NC], f32)
        CM = const.tile([P, P], bf)
        ID = const.tile([P, P], bf)
        ONE = const.tile([P, 1], bf)

        nc.sync.dma_start(out=WQ, in_=wq_d.rearrange("kb p n -> p kb n"))
        nc.sync.dma_start(out=WK, in_=wk_d.rearrange("kb p n -> p kb n"))
        nc.sync.dma_start(out=WV, in_=wv_d.rearrange("kb p n -> p kb n"))
        nc.sync.dma_start(out=WD, in_=wd_d.rearrange("kb p n -> p kb n"))
        nc.sync.dma_start(out=GC, in_=gcol_d)
        nc.sync.dma_start(out=OMG, in_=omg_d)
        nc.sync.dma_start(out=ZM, in_=zmask_d)
        nc.sync.dma_start(out=OH, in_=oh_d)
        nc.sync.dma_start(out=CM, in_=mask_d)
        nc.sync.dma_start(out=ID, in_=ident_d)
        nc.vector.memset(ONE, 1.0)

        # collective bounce buffers
        cs_in = dram.tile([BS, HD], f32)
        cs_out = dram.tile([NC * BS, HD], f32)
        ab_in = dram.tile([2, HD, HD], bf)
        zrow_d = dram.tile([BS, HD], bf)
        rca_d = dram.tile([BS, H, SEG], bf)
        rcm_d = dram.tile([BS, SEG], bf)
        ab_out = dram.tile([NC, 2, HD, HD], bf)

        # retained across phases (bufs=BS -> one slot per batch-segment)
        skT = [keep.tile([P, NB, HD], bf, tag="sk", name=f"sk{i}") for i in range(BS)]
        sqT = [keep.tile([P, NB, SEG], bf, tag="sq", name=f"sq{i}") for i in range(BS)]
        step_d = dram.tile([BS, NB, P, SEG], bf)  # attention-term scratch

        # z tiles (phase boundary singles)
        ZROW = phb.tile([BS, HD], f32)      # z at segment start, row form
        ZCOL = phb.tile([P, NB, BS], bf)    # column form for denominators
        AT0 = phb.tile([P, NB, HD], bf)     # segment-0 A-part (retained)
        BT0 = phb.tile([P, NB, HD], bf)
        MSEL = phb.tile([P, NB, HD], bf)    # selected M at segment 2c
        MLOC1 = phb.tile([P, NB, HD], bf)   # M at segment 2c+1

        def bs_of(b, j):
            return j * B + b

        # ============ PHASE A1: k-projection, sk, colsums ============
        with tc.tile_pool(name="pa1", bufs=2) as pa1, \
             tc.tile_pool(name="ps1", bufs=2, space="PSUM") as ps1:
            for j in range(SPC):
                for b in range(B):
                    i = bs_of(b, j)
                    XT = pa1.tile([P, NB, SEG], bf, tag="xt")
                    nc.sync.dma_start(out=XT, in_=xt_d[b, j].rearrange("kb p s -> p kb s"))
                    sk_i = skT[i]
                    for sb in range(NB):
                        pk = ps1.tile([P, SEG], f32, tag="pk")
                        for kb in range(NB):
                            nc.tensor.matmul(
                                pk,
                                lhsT=XT[:, kb, sb * P:(sb + 1) * P],
                                rhs=WK[:, kb, :],
                                start=(kb == 0),
                                stop=(kb == NB - 1),
                            )
                        # elu1(k) = max(k + 1, exp(min(k, 0)))
                        em = pa1.tile([P, SEG], bf, tag="em")
                        nc.vector.tensor_scalar_min(em, pk, 0.0)
                        ee = pa1.tile([P, SEG], bf, tag="ee")
                        nc.scalar.activation(ee, em, AF.Exp)
                        nc.vector.scalar_tensor_tensor(
                            out=sk_i[:, sb, :], in0=pk, scalar=1.0, in1=ee,
                            op0=OP.add, op1=OP.max,
                        )
                    pc = ps1.tile([1, HD], f32, tag="pc")
                    for sb in range(NB):
                        nc.tensor.matmul(
                            pc, lhsT=ONE, rhs=sk_i[:, sb, :],
                            start=(sb == 0), stop=(sb == NB - 1),
                        )
                    cs_sb = pa1.tile([1, HD], f32, tag="cs")
                    nc.scalar.activation(cs_sb, pc, AF.Copy)
                    nc.sync.dma_start(out=cs_in[i:i + 1, :], in_=cs_sb)

        if STAGE >= 2:
            # ============ AG1: colsums ============
            nc.gpsimd.collective_compute(
                "AllGather", OP.bypass,
                replica_groups=[list(range(NC))],
                ins=[cs_in.opt()], outs=[cs_out.opt()],
            )

            # ============ z prefix ============
            with tc.tile_pool(name="pz", bufs=1) as pz, \
                 tc.tile_pool(name="psz", bufs=1, space="PSUM") as psz:
                Z = pz.tile([NC * BS, HD], f32, tag="z")
                nc.sync.dma_start(out=Z, in_=cs_out)
                zp = psz.tile([BS, HD], f32, tag="zp")
                nc.tensor.matmul(zp, lhsT=ZM, rhs=Z, start=True, stop=True)
                nc.scalar.activation(ZROW, zp, AF.Copy, bias=1.0 / D)
                ZROW16 = pz.tile([BS, HD], bf, tag="zr16")
                nc.vector.tensor_copy(ZROW16, ZROW)
                nc.sync.dma_start(out=zrow_d, in_=ZROW16)
                for kb in range(NB):
                    zc = psz.tile([P, BS], f32, tag="zc")
                    nc.tensor.matmul(zc, lhsT=Z[:, kb * P:(kb + 1) * P], rhs=ZM,
                                     start=True, stop=True)
                    nc.scalar.activation(ZCOL[:, kb, :], zc, AF.Copy, bias=1.0 / D)


        if STAGE >= 3:
            # ============ PHASE A2: per-segment q/kT/v proj, attention, d/skd, A/B ============
            with tc.tile_pool(name="pa2", bufs=2) as pa2, \
                 tc.tile_pool(name="pskd", bufs=B) as pskd, \
                 tc.tile_pool(name="pva", bufs=B) as pva, \
                 tc.tile_pool(name="pw", bufs=3) as pw, \
                 tc.tile_pool(name="pab", bufs=1) as pab, \
                 tc.tile_pool(name="ps2", bufs=2, space="PSUM") as ps2, \
                 tc.tile_pool(name="psc", bufs=2, space="PSUM") as psc, \
                 tc.tile_pool(name="psa", bufs=2, space="PSUM") as psa:
                at1 = bt1 = None
                for j in range(SPC):
                    skd = [None] * B
                    vaug = [None] * B
                    for b in range(B):
                        i = bs_of(b, j)
                        XT = pa2.tile([P, NB, SEG], bf, tag="xt")
                        nc.sync.dma_start(out=XT, in_=xt_d[b, j].rearrange("kb p s -> p kb s"))

                        # --- qT (transposed: hd on partitions) ---
                        qh = pa2.tile([P, NB, SEG], bf, tag="qh")
                        sq_i = sqT[i]
                        for mb in range(NB):
                            pq = ps2.tile([P, SEG], f32, tag="pp")
                            for kb in range(NB):
                                nc.tensor.matmul(
                                    pq, lhsT=WQ[:, kb, mb * P:(mb + 1) * P],
                                    rhs=XT[:, kb, :],
                                    start=(kb == 0), stop=(kb == NB - 1),
                                )
                            nc.scalar.activation(qh[:, mb, :], pq, AF.Copy)
                            em = pa2.tile([P, SEG], bf, tag="em")
                            nc.vector.tensor_scalar_min(em, pq, 0.0)
                            ee = pa2.tile([P, SEG], bf, tag="ee")
                            nc.scalar.activation(ee, em, AF.Exp)
                            nc.vector.scalar_tensor_tensor(
                                out=sq_i[:, mb, :], in0=pq, scalar=1.0, in1=ee,
                                op0=OP.add, op1=OP.max,
                            )
                        # --- kT ---
                        kh = pa2.tile([P, NB, SEG], bf, tag="kh", bufs=1)
                        for mb in range(NB):
                            pkt = ps2.tile([P, SEG], f32, tag="pp")
                            for kb in range(NB):
                                nc.tensor.matmul(
                                    pkt, lhsT=WK[:, kb, mb * P:(mb + 1) * P],
                                    rhs=XT[:, kb, :],
                                    start=(kb == 0), stop=(kb == NB - 1),
                                )
                            nc.scalar.activation(kh[:, mb, :], pkt, AF.Copy)
                        # --- v (original orientation) + aug ones column ---
                        va = pva.tile([P, NB, H, D + 1], bf, tag="va")
                        vaug[b] = va
                        nc.vector.memset(va[:, :, :, D:D + 1], 1.0)
                        for sb in range(NB):
                            pv = ps2.tile([P, SEG], f32, tag="pp")
                            for kb in range(NB):
                                nc.tensor.matmul(
                                    pv, lhsT=XT[:, kb, sb * P:(sb + 1) * P],
                                    rhs=WV[:, kb, :],
                                    start=(kb == 0), stop=(kb == NB - 1),
                                )
                            nc.vector.tensor_copy(
                                va[:, sb, :, 0:D], pv.rearrange("p (h d) -> p h d", h=H)
                            )

                        if SUB >= 2:
                            # --- attention ---
                            st_i = pa2.tile([P, NB, SEG], bf, tag="stp", name=f"stp{i}")
                            for h in range(H):
                                hb, ho = h // 2, (h % 2) * 64
                                pat = psa.tile([D + 1, SEG], f32, tag="at")
                                for kb in range(NB):
                                    q0 = kb * P
                                    qf = SEG - q0
                                    ps_ = psc.tile([P, SEG], f32, tag="sc")
                                    nc.tensor.matmul(
                                        ps_[:, 0:qf],
                                        lhsT=kh[ho:ho + 64, hb, q0:q0 + P],
                                        rhs=qh[ho:ho + 64, hb, q0:SEG],
                                        start=True, stop=True,
                                    )
                                    wt = pw.tile([P, SEG], bf, tag="wt")
                                    nc.scalar.activation(wt[:, 0:qf], ps_[:, 0:qf], AF.Exp,
                                                         scale=0.125)
                                    # causal mask on the diagonal 128x128 block
                                    nc.vector.tensor_mul(wt[:, 0:P], wt[:, 0:P], CM)
                                    nc.tensor.matmul(
                                        pat[:, q0:SEG],
                                        lhsT=va[:, kb, h, :],
                                        rhs=wt[:, 0:qf],
                                        start=(kb == 0), stop=(kb == NB - 1),
                                    )
                                rca = pw.tile([1, SEG], bf, tag="rca")
                                with nc.allow_low_precision(reason="bf16 softmax recip"):
                                    nc.vector.reciprocal(rca, pat[D:D + 1, :])
                                nc.sync.dma_start(out=rca_d[i, h], in_=rca)
                                rcab = pw.tile([P, SEG], bf, tag="rcab")
                                nc.gpsimd.dma_start(
                                    out=rcab[ho:ho + D, :],
                                    in_=rca_d[i:i + 1, h, :].partition_broadcast(D))
                                nc.vector.scalar_tensor_tensor(
                                    out=st_i[ho:ho + 64, hb, :],
                                    in0=pat[0:D, :],
                                    scalar=OMG[ho:ho + 64, hb:hb + 1],
                                    in1=rcab[ho:ho + D, :],
                                    op0=OP.mult, op1=OP.mult,
                                )

                            nc.sync.dma_start(
                                out=step_d[i].rearrange("kb p s -> p kb s"), in_=st_i)


                        if SUB >= 3:
                            # --- d and sk/d ---
                            i_row = bs_of(b, j)
                            sk_i = skT[i]
                            sd = pskd.tile([P, NB, HD], bf, tag="skd")
                            skd[b] = sd
                            dcol = pa2.tile([P, NB], f32, tag="d")
                            rcd = pa2.tile([P, NB], f32, tag="rcd")
                            jnk = pa2.tile([P, HD], bf, tag="jnk", bufs=1)
                            zbp = pa2.tile([P, HD], bf, tag="zbp")
                            nc.gpsimd.dma_start(
                                out=zbp,
                                in_=zrow_d[i_row:i_row + 1, :].partition_broadcast(P))
                            for sb in range(NB):
                                nc.vector.tensor_mul(jnk, sk_i[:, sb, :], zbp)
                                nc.vector.tensor_reduce(
                                    out=dcol[:, sb:sb + 1], in_=jnk,
                                    axis=mybir.AxisListType.X, op=OP.add,
                                )
                            nc.vector.reciprocal(rcd, dcol)
                            for sb in range(NB):
                                nc.vector.tensor_scalar_mul(
                                    sd[:, sb, :], sk_i[:, sb, :], rcd[:, sb:sb + 1]
                                )


                    if SUB >= 4:
                        # --- A_t, B_t for this segment (sum over batches) ---
                        at_t = pab.tile([P, NB, HD], bf, tag="at", name=f"at{j}") if j > 0 else AT0
                        bt_t = pab.tile([P, NB, HD], bf, tag="bt", name=f"bt{j}") if j > 0 else BT0
                        for mb in range(NB):
                            pA = ps2.tile([P, HD], f32, tag="pp")
                            n = 0
                            for b in range(B):
                                for sb in range(NB):
                                    nc.tensor.matmul(
                                        pA,
                                        lhsT=skT[bs_of(b, j)][:, sb, mb * P:(mb + 1) * P],
                                        rhs=skd[b][:, sb, :],
                                        start=(n == 0), stop=(n == B * NB - 1),
                                    )
                                    n += 1
                            # negate: A-part = -K
                            nc.scalar.activation(at_t[:, mb, :], pA, AF.Copy, scale=-1.0)
                        for mb in range(NB):
                            pB = ps2.tile([P, HD], f32, tag="pp")
                            n = 0
                            for b in range(B):
                                for sb in range(NB):
                                    nc.tensor.matmul(
                                        pB.rearrange("p (h d) -> p h d", h=H),
                                        lhsT=skT[bs_of(b, j)][:, sb, mb * P:(mb + 1) * P],
                                        rhs=vaug[b][:, sb, :, 0:D],
                                        start=(n == 0), stop=(n == B * NB - 1),
                                    )
                                    n += 1
                            nc.scalar.activation(bt_t[:, mb, :], pB, AF.Copy)
                        if j > 0:
                            at1, bt1 = at_t, bt_t


                if SUB >= 5:
                    # --- pair composition: abA = Abar^T = A0 A1 + A0 + A1 ; abB = Bbar ---
                    abA = pab.tile([P, NB, HD], bf, tag="abA")
                    abB = pab.tile([P, NB, HD], bf, tag="abB")
                    for mb in range(NB):
                        pA = ps2.tile([P, HD], f32, tag="pp")
                        for kb in range(NB):
                            nc.tensor.matmul(
                                pA, lhsT=AT0[:, kb, mb * P:(mb + 1) * P], rhs=at1[:, kb, :],
                                start=(kb == 0), stop=False,
                            )
                        nc.tensor.matmul(pA, lhsT=ID, rhs=AT0[:, mb, :], start=False, stop=False)
                        nc.tensor.matmul(pA, lhsT=ID, rhs=at1[:, mb, :], start=False, stop=True)
                        nc.scalar.activation(abA[:, mb, :], pA, AF.Copy)
                    for mb in range(NB):
                        pB = ps2.tile([P, HD], f32, tag="pp")
                        for kb in range(NB):
                            nc.tensor.matmul(
                                pB, lhsT=at1[:, kb, mb * P:(mb + 1) * P], rhs=BT0[:, kb, :],
                                start=(kb == 0), stop=False,
                            )
                        nc.tensor.matmul(pB, lhsT=ID, rhs=BT0[:, mb, :], start=False, stop=False)
                        nc.tensor.matmul(pB, lhsT=ID, rhs=bt1[:, mb, :], start=False, stop=True)
                        nc.scalar.activation(abB[:, mb, :], pB, AF.Copy)
                    nc.sync.dma_start(out=ab_in[0].rearrange("(kb p) n -> p kb n", p=P), in_=abA)
                    nc.sync.dma_start(out=ab_in[1].rearrange("(kb p) n -> p kb n", p=P), in_=abB)



        if STAGE >= 5:
            # ============ AG2: pair compositions ============
            nc.gpsimd.collective_compute(
                "AllGather", OP.bypass,
                replica_groups=[list(range(NC))],
                ins=[ab_in.opt()], outs=[ab_out.opt()],
            )


        if STAGE >= 6:
            # ============ chain + select ============
            nc.vector.memset(MSEL, 0.0)
            with tc.tile_pool(name="pch", bufs=2) as pch, \
                 tc.tile_pool(name="psch", bufs=NB, space="PSUM") as psch:
                pM = [psch.tile([P, HD], f32, tag="ch", name=f"chain{i}") for i in range(NB)]
                mprev = None
                for step in range(NC - 1):
                    cA = pch.tile([P, NB, HD], bf, tag="cA")
                    cB = pch.tile([P, NB, HD], bf, tag="cB")
                    nc.sync.dma_start(
                        out=cA, in_=ab_out[step, 0].rearrange("(kb p) n -> p kb n", p=P))
                    nc.sync.dma_start(
                        out=cB, in_=ab_out[step, 1].rearrange("(kb p) n -> p kb n", p=P))
                    mcur = pch.tile([P, NB, HD], bf, tag="mc")
                    for mb in range(NB):
                        if step == 0:
                            nc.tensor.matmul(pM[mb], lhsT=ID, rhs=cB[:, mb, :],
                                             start=True, stop=True)
                        elif SIMSAFE:
                            for kb in range(NB):
                                nc.tensor.matmul(
                                    pM[mb], lhsT=cA[:, kb, mb * P:(mb + 1) * P],
                                    rhs=mprev[:, kb, :],
                                    start=(kb == 0), stop=False,
                                )
                            nc.tensor.matmul(pM[mb], lhsT=ID, rhs=mprev[:, mb, :],
                                             start=False, stop=False)
                            nc.tensor.matmul(pM[mb], lhsT=ID, rhs=cB[:, mb, :],
                                             start=False, stop=True)
                        else:
                            for kb in range(NB):
                                nc.tensor.matmul(
                                    pM[mb], lhsT=cA[:, kb, mb * P:(mb + 1) * P],
                                    rhs=mprev[:, kb, :],
                                    start=False, stop=False,
                                )
                            nc.tensor.matmul(pM[mb], lhsT=ID, rhs=cB[:, mb, :],
                                             start=False, stop=True)
                        nc.scalar.activation(mcur[:, mb, :], pM[mb], AF.Copy)
                        nc.vector.scalar_tensor_tensor(
                            out=MSEL[:, mb, :], in0=mcur[:, mb, :],
                            scalar=OH[:, step:step + 1], in1=MSEL[:, mb, :],
                            op0=OP.mult, op1=OP.add,
                        )
                    mprev = mcur


        if STAGE >= 7:
            # ============ phase B: M_loc1, mem_ret, combine, Wd ============
            with tc.tile_pool(name="pb", bufs=2) as pb, \
                 tc.tile_pool(name="psb", bufs=2, space="PSUM") as psb, \
                 tc.tile_pool(name="psw", bufs=2, space="PSUM") as psw:
                # M at segment 2c+1 = M + A0-part @ M + B0
                for mb in range(NB):
                    pm = psb.tile([P, HD], f32, tag="mm")
                    for kb in range(NB):
                        nc.tensor.matmul(
                            pm, lhsT=AT0[:, kb, mb * P:(mb + 1) * P], rhs=MSEL[:, kb, :],
                            start=(kb == 0), stop=False,
                        )
                    nc.tensor.matmul(pm, lhsT=ID, rhs=MSEL[:, mb, :], start=False, stop=False)
                    nc.tensor.matmul(pm, lhsT=ID, rhs=BT0[:, mb, :], start=False, stop=True)
                    nc.scalar.activation(MLOC1[:, mb, :], pm, AF.Copy)

                for j in range(SPC):
                    Mt = MSEL if j == 0 else MLOC1
                    for b in range(B):
                        i = bs_of(b, j)
                        st_i = pb.tile([P, NB, SEG], bf, tag="stp2", name=f"stp2_{i}")
                        nc.sync.dma_start(
                            out=st_i, in_=step_d[i].rearrange("kb p s -> p kb s"))
                        sq_i = sqT[i]
                        # denominator 1 x SEG
                        pd = psw.tile([1, SEG], f32, tag="dn")
                        for kb in range(NB):
                            nc.tensor.matmul(
                                pd, lhsT=ZCOL[:, kb, i:i + 1], rhs=sq_i[:, kb, :],
                                start=(kb == 0), stop=(kb == NB - 1),
                            )
                        rcm = pb.tile([1, SEG], bf, tag="rcm")
                        with nc.allow_low_precision(reason="bf16 memread recip"):
                            nc.vector.reciprocal(rcm, pd)
                        nc.sync.dma_start(out=rcm_d[i], in_=rcm)
                        rcmb = pb.tile([P, SEG], bf, tag="rcmb")
                        nc.gpsimd.dma_start(
                            out=rcmb,
                            in_=rcm_d[i:i + 1, :].partition_broadcast(P))
                        for mb in range(NB):
                            pm = psb.tile([P, SEG], f32, tag="mm")
                            for kb in range(NB):
                                nc.tensor.matmul(
                                    pm, lhsT=Mt[:, kb, mb * P:(mb + 1) * P],
                                    rhs=sq_i[:, kb, :],
                                    start=(kb == 0), stop=(kb == NB - 1),
                                )
                            mtmp = pb.tile([P, SEG], bf, tag="mt")
                            nc.vector.scalar_tensor_tensor(
                                out=mtmp, in0=pm, scalar=GC[:, mb:mb + 1],
                                in1=rcmb,
                                op0=OP.mult, op1=OP.mult,
                            )
                            nc.vector.tensor_add(st_i[:, mb, :], st_i[:, mb, :], mtmp)
                        for sb in range(NB):
                            po = psw.tile([P, D], f32, tag="wd")
                            for mb in range(NB):
                                nc.tensor.matmul(
                                    po, lhsT=st_i[:, mb, sb * P:(sb + 1) * P],
                                    rhs=WD[:, mb, :],
                                    start=(mb == 0), stop=(mb == NB - 1),
                                )
                            ob = pb.tile([P, D], f32, tag="ob")
                            nc.scalar.activation(ob, po, AF.Copy)
                            nc.sync.dma_start(
                                out=out_d[b, j, sb * P:(sb + 1) * P, :], in_=ob)


    nc.compile()
    return nc


def _prep_inputs(x, Wq, Wk, Wv, Wd, beta):
    """Host-side prep: transpose/cast/shard. Returns in_maps (list of 8 dicts)."""
    g = 1.0 / (1.0 + np.exp(-beta.astype(np.float64)))  # (H,)
    g = g.astype(np.float32)
    gcol = np.repeat(g, D).reshape(NB, P).T.copy()      # (P, NB): g[(kb*128+p)//64]
    omg = (1.0 - np.repeat(g, D)).reshape(NB, P).T.copy()

    def wprep(w):
        return np.ascontiguousarray(
            w.reshape(NB, P, w.shape[1]).astype(bf_np))

    wq_a, wk_a, wv_a = wprep(Wq), wprep(Wk), wprep(Wv)
    wd_a = wprep(Wd)
    cmask = np.triu(np.ones((P, P), np.float32)).astype(bf_np)
    ident = np.eye(P, dtype=np.float32).astype(bf_np)

    # x -> per-core transposed blocks: xt[b, j, kb, p, s] = x[b, (2c+j)*SEG+s, kb*P+p]
    xs = x.reshape(B, NSEG, SEG, DIN)
    in_maps = []
    for c in range(NC):
        xloc = xs[:, 2 * c:2 * c + 2]                        # (B, SPC, SEG, DIN)
        xt = xloc.transpose(0, 1, 3, 2)                      # (B, SPC, DIN, SEG)
        xt = np.ascontiguousarray(
            xt.reshape(B, SPC, NB, P, SEG).astype(bf_np))
        # AG1 global row for (t, b): rank t//2 contributes row (t%2)*B + b
        zmask = np.zeros((64, NC), np.float32)
        for jj in range(NC):
            tgt = 2 * c + (jj // B)
            bb = jj % B
            for t in range(NSEG):
                if t < tgt:
                    zmask[(t // 2) * BS + (t % 2) * B + bb, jj] = 1.0
        oh = np.zeros((P, NC), np.float32)
        if c >= 1:
            oh[:, c - 1] = 1.0
        in_maps.append({
            "xt": xt, "wq": wq_a, "wk": wk_a, "wv": wv_a, "wd": wd_a,
            "gcol": gcol, "omg": omg, "zmask": zmask, "oh": oh,
            "cmask": cmask, "ident": ident,
        })
    return in_maps


def kernel(x, Wq, Wk, Wv, Wd, beta, _trace=False):
    x = np.asarray(x, np.float32)
    in_maps = _prep_inputs(
        x, np.asarray(Wq, np.float32), np.asarray(Wk, np.float32),
        np.asarray(Wv, np.float32), np.asarray(Wd, np.float32),
        np.asarray(beta, np.float32))
    if "nc" not in _CACHE:
        _CACHE["nc"] = _build()
    nc = _CACHE["nc"]
    res = bass_utils.run_bass_kernel_spmd(
        nc, in_maps, core_ids=list(range(NC)), trace=_trace)
    _CACHE["last_results"] = res
    out = np.empty((B, L, D), np.float32)
    for c in range(NC):
        oc = res.results[c]["out"]                  # (B, SPC, SEG, D)
        out[:, 2 * c * SEG:(2 * c + 2) * SEG, :] = oc.reshape(B, SPC * SEG, D)
    return out

